# revision 26
# baseline (speedup 1.0000x reference)
"""Multi-head attention with additive positional bias on 8 Trainium2 cores.

Problem: q,k,v [8, 1024, 512] fp32, pos_bias [1, 8, 1024, 1024] fp32,
8 heads x head_dim 64, out = softmax(q@k^T * scale + bias) @ v.

Sharding: one head per NeuronCore (tensor parallel over heads). The bias
table is per-head, so each core only needs its own bias slice.

Per-core pipeline (scores transposed: S^T[j,i], j on partitions):
  MM1:  ps[j,i] = A*scale*(K_tile^T Q)   (A = 128/ln2 folded into Q on host)
  exp:  tiles are split across two engines so neither is the bottleneck:
    - 5 of 8 j-tiles on ACT: exp(ps/A) -> bf16, then Pool multiplies by
      the resident exp(bias^T) table.
    - 3 of 8 j-tiles on DVE via a Schraudolph exp: one scalar_tensor_tensor
      (ps + (16256-C)) + round(A*bias^T)  -> int16, whose bit pattern IS
      bf16(exp(s+bias)) to ~2%; softmax normalization cancels the uniform
      component, leaving ~1.1e-2 rel error overall (validated in numpy).
  MM2:  po[dv,i] += Vpad_tile^T P^T, ones-column gives denominators free.
PE runs a skew-2 software pipeline (MM1(t+2) issued ahead of MM2(t)) over
3 PSUM score slots so it never waits on ACT/DVE. Warm-up matmuls during
the initial DMA keep the HAM clock gate from throttling the real work.
Host does the final divide/untranspose.
"""

import numpy as np
from contextlib import ExitStack

import concourse.bacc as bacc
import concourse.bass as bass
import concourse.mybir as mybir
import concourse.tile as tile
from concourse.bass_utils import run_bass_kernel_spmd

B = 8          # batch
S = 1024       # sequence length
D = 512        # model dim
H = 8          # heads
HD = 64        # head dim
NT = S // 128  # 128-row j-tiles per sequence
SCALE = HD ** -0.5
A16 = 128.0 / float(np.log(2.0))   # maps scores into bf16 exponent code space
C16 = 7.0                          # Schraudolph correction (tuned numerically)
D_TILES = (0, 2, 5, 7)            # j-tiles exp'd on DVE (Schraudolph)
A_TILES = (1, 3, 4, 6)             # j-tiles exp'd on ACT
N_WARMUP = 8                       # PE warm-up matmuls (HAM clock gate)

_PROGRAM = None


def _emit(ctx, tc, out, qt, kt, vp, eba, abd):
    nc = tc.nc
    f32 = mybir.dt.float32
    bf16 = mybir.dt.bfloat16
    i16 = mybir.dt.int16
    a_slot = {t: i for i, t in enumerate(A_TILES)}
    d_slot = {t: i for i, t in enumerate(D_TILES)}

    singles = ctx.enter_context(tc.tile_pool(name="singles", bufs=1))
    qk_pool = ctx.enter_context(tc.tile_pool(name="qk_pool", bufs=2))
    v_pool = ctx.enter_context(tc.tile_pool(name="v_pool", bufs=2))
    e_pool = ctx.enter_context(tc.tile_pool(name="e_pool", bufs=3))
    # pbf(0) is consumed only at the end of its batch (rotated MM2 order), so
    # keep enough bufs that no same-batch allocation ever WAR-waits on it.
    p_pool = ctx.enter_context(tc.tile_pool(name="p_pool", bufs=10))
    o_pool = ctx.enter_context(tc.tile_pool(name="o_pool", bufs=2))
    ps_s = ctx.enter_context(tc.tile_pool(name="ps_s", bufs=3, space="PSUM"))
    ps_o = ctx.enter_context(tc.tile_pool(name="ps_o", bufs=1, space="PSUM"))

    # First batch's operands on the Sync DMA queue; the resident bias tables
    # on the Activation engine's DMA queue (idle until the first exp) so the
    # ~0.6us/DMA descriptor generation runs in parallel with Sync's.
    qtb = qk_pool.tile([128, S], bf16, tag="qtb")
    ktb = qk_pool.tile([128, S], bf16, tag="ktb")
    vpb = v_pool.tile([128, NT, HD + 1], bf16, tag="vpb")
    nc.sync.dma_start(out=ktb, in_=kt[0])
    nc.sync.dma_start(out=qtb, in_=qt[0])
    nc.sync.dma_start(out=vpb, in_=vp[0])

    # Bias tables: exp(bias^T) bf16 for ACT tiles, round(A*bias^T) int16 for
    # DVE tiles. Partition-major DRAM layout; DMA'd in first-use order
    # (j-tile order 0A 1D 2A 3A 4D 5A 6A 7D) so batch 0 never waits.
    ebt_all = singles.tile([128, len(A_TILES), S], bf16, name="ebt_all")
    abt_all = singles.tile([128, len(D_TILES), S], i16, name="abt_all")
    nc.sync.dma_start(out=abt_all[:, 0:1, :], in_=abd[:, 0:1, :])
    nc.sync.dma_start(out=ebt_all[:, 0:1, :], in_=eba[:, 0:1, :])
    nc.sync.dma_start(out=abt_all[:, 1:2, :], in_=abd[:, 1:2, :])
    nc.sync.dma_start(out=ebt_all[:, 1:3, :], in_=eba[:, 1:3, :])
    nc.sync.dma_start(out=abt_all[:, 2:3, :], in_=abd[:, 2:3, :])
    nc.sync.dma_start(out=ebt_all[:, 3:4, :], in_=eba[:, 3:4, :])
    nc.sync.dma_start(out=abt_all[:, 3:4, :], in_=abd[:, 3:4, :])
    eb_tiles = {t: ebt_all[:, i, :] for i, t in enumerate(A_TILES)}
    ab_tiles = {t: abt_all[:, i, :] for i, t in enumerate(D_TILES)}

    # Warm-up matmuls: keep the PE busy during the DMA head so the HAM
    # clock gate reaches 2.4 GHz before the real matmuls start.
    wrm = singles.tile([128, 512], bf16, name="wrm")
    nc.gpsimd.memset(wrm, 0)
    wps = ps_s.tile([128, S], f32, tag="ps")
    for _ in range(N_WARMUP):
        nc.tensor.matmul(wps[:, 0:512], wrm[:, 0:128], wrm, start=True, stop=True)

    # Per-batch skew-2 software pipeline: MM1(t) | exp(t-1) | MM2(t-2).
    for b in range(B):
        if b > 0:
            qtb = qk_pool.tile([128, S], bf16, tag="qtb", name=f"qtb{b}")
            nc.sync.dma_start(out=qtb, in_=qt[b])
            ktb = qk_pool.tile([128, S], bf16, tag="ktb", name=f"ktb{b}")
            nc.sync.dma_start(out=ktb, in_=kt[b])
            vpb = v_pool.tile([128, NT, HD + 1], bf16, tag="vpb", name=f"vpb{b}")
            nc.sync.dma_start(out=vpb, in_=vp[b])

        po = ps_o.tile([HD + 1, S], f32, tag="po")
        ps_tiles = [None] * NT
        pbf_tiles = [None] * NT
        # Tiles 1 and 3 mul on the slow GpSimd engine, so their MM2s are
        # rotated to the end of the accumulation group (order below); tile 0
        # opens the group (start), tile 3 closes it (stop).
        mm2_order = [0, 2, 4, 5, 6, 7, 1, 3]
        for ph in range(NT + 3):
            # stage 1: scores for tile t
            t = ph
            if t < NT:
                ps = ps_s.tile([128, S], f32, tag="ps")
                ps_tiles[t] = ps
                for c in range(2):
                    cs = slice(c * 512, (c + 1) * 512)
                    nc.tensor.matmul(
                        ps[:, cs],
                        ktb[:, t * 128:(t + 1) * 128],
                        qtb[:, cs],
                        start=True,
                        stop=True,
                    )
            # stage 2: exp for tile u
            u = ph - 1
            if 0 <= u < NT:
                ps = ps_tiles[u]
                pbf = p_pool.tile([128, S], bf16, tag="pbf")
                pbf_tiles[u] = pbf
                if u in d_slot:
                    # pbf bits = int16((ps + 16256-C) + round(A*bias))
                    nc.vector.scalar_tensor_tensor(
                        pbf.bitcast(i16),
                        ps,
                        float(16256.0 - C16),
                        ab_tiles[u],
                        mybir.AluOpType.add,
                        mybir.AluOpType.add,
                    )
                else:
                    ebf = e_pool.tile([128, S], bf16, tag="ebf")
                    nc.scalar.activation(
                        ebf, ps, mybir.ActivationFunctionType.Exp,
                        scale=float(1.0 / A16),
                    )
                    # GpSimd's mul is 2.6x slower than DVE's: it gets tiles
                    # 1 and 3, whose MM2s are deferred to the batch end.
                    eng = nc.gpsimd if u in (1, 3) else nc.vector
                    eng.tensor_mul(pbf, ebf, eb_tiles[u])
            # stage 3: PV accumulation, rotated order
            if ph >= 3:
                w = mm2_order[ph - 3]
                pbf = pbf_tiles[w]
                for c in range(2):
                    cs = slice(c * 512, (c + 1) * 512)
                    nc.tensor.matmul(
                        po[:, cs],
                        vpb[:, w, :],
                        pbf[:, cs],
                        start=(w == 0),
                        stop=(w == 3),
                    )
        osb = o_pool.tile([HD + 1, S], f32, tag="osb")
        nc.scalar.copy(osb, po)
        nc.sync.dma_start(out=out[b], in_=osb)


def _build_program():
    nc = bacc.Bacc("TRN2", target_bir_lowering=False, debug=False)
    qt = nc.dram_tensor("qt", [B, 128, S], mybir.dt.bfloat16, kind="ExternalInput").ap()
    kt = nc.dram_tensor("kt", [B, 128, S], mybir.dt.bfloat16, kind="ExternalInput").ap()
    vp = nc.dram_tensor(
        "vp", [B, 128, NT, HD + 1], mybir.dt.bfloat16, kind="ExternalInput"
    ).ap()
    eba = nc.dram_tensor(
        "eba", [128, len(A_TILES), S], mybir.dt.bfloat16, kind="ExternalInput"
    ).ap()
    abd = nc.dram_tensor(
        "abd", [128, len(D_TILES), S], mybir.dt.int16, kind="ExternalInput"
    ).ap()
    out = nc.dram_tensor("out", [B, HD + 1, S], mybir.dt.float32, kind="ExternalOutput").ap()
    with tile.TileContext(nc) as tc, ExitStack() as ctx:
        _emit(ctx, tc, out, qt, kt, vp, eba, abd)
    nc.compile()
    return nc


def get_program():
    global _PROGRAM
    if _PROGRAM is None:
        _PROGRAM = _build_program()
    return _PROGRAM


def make_in_maps(q, k, v, pos_bias):
    import ml_dtypes

    bf16 = ml_dtypes.bfloat16
    q4 = q.reshape(B, S, H, HD)
    k4 = k.reshape(B, S, H, HD)
    v4 = v.reshape(B, S, H, HD)
    ones = np.ones((B, S, 1), np.float32)
    in_maps = []
    for h in range(H):
        qt = np.zeros((B, 128, S), bf16)
        qt[:, :HD, :] = (
            q4[:, :, h, :].transpose(0, 2, 1) * np.float32(SCALE * A16)
        ).astype(bf16)
        kt = np.zeros((B, 128, S), bf16)
        kt[:, :HD, :] = k4[:, :, h, :].transpose(0, 2, 1).astype(bf16)
        vpf = np.concatenate([v4[:, :, h, :], ones], axis=2)  # [B, S, 65]
        vpf = np.ascontiguousarray(
            vpf.reshape(B, NT, 128, HD + 1).transpose(0, 2, 1, 3)
        ).astype(bf16)  # [B, 128, NT, 65]
        bT = pos_bias[0, h].T  # [j, i]
        # [128(j%128), n_tiles, S(i)] partition-major for single-DMA loads
        eba = np.ascontiguousarray(
            np.stack(
                [np.exp(bT[t * 128:(t + 1) * 128]).astype(bf16) for t in A_TILES]
            ).transpose(1, 0, 2)
        )
        abd = np.ascontiguousarray(
            np.stack(
                [
                    np.rint(bT[t * 128:(t + 1) * 128] * A16).astype(np.int16)
                    for t in D_TILES
                ]
            ).transpose(1, 0, 2)
        )
        in_maps.append({"qt": qt, "kt": kt, "vp": vpf, "eba": eba, "abd": abd})
    return in_maps


def assemble_output(results):
    out = np.empty((B, S, D), np.float32)
    for h in range(H):
        o = results[h]["out"]  # [B, 65, S]
        normed = o[:, :HD, :] / o[:, HD:HD + 1, :]
        out[:, :, h * HD:(h + 1) * HD] = normed.transpose(0, 2, 1)
    return out


def kernel(q, k, v, pos_bias):
    nc = get_program()
    in_maps = make_in_maps(
        np.asarray(q, np.float32),
        np.asarray(k, np.float32),
        np.asarray(v, np.float32),
        np.asarray(pos_bias, np.float32),
    )
    res = run_bass_kernel_spmd(nc, in_maps, list(range(H))).results
    return assemble_output(res)


# revision 27
# speedup vs baseline: 1.1177x; 1.1177x over previous
"""Multi-head attention with additive positional bias on 8 Trainium2 cores.

Problem: q,k,v [8, 1024, 512] fp32, pos_bias [1, 8, 1024, 1024] fp32,
8 heads x head_dim 64, out = softmax(q@k^T * scale + bias) @ v.

Sharding: one head per NeuronCore (tensor parallel over heads). The bias
table is per-head, so each core only needs its own bias slice.

Per-core pipeline (scores transposed: S^T[j,i], j on partitions):
  MM1:  ps[j,i] = A*scale*(K_tile^T Q)   (A = 128/ln2 folded into Q on host)
  exp:  tiles are split across two engines so neither is the bottleneck:
    - 5 of 8 j-tiles on ACT: exp(ps/A) -> bf16, then Pool multiplies by
      the resident exp(bias^T) table.
    - 3 of 8 j-tiles on DVE via a Schraudolph exp: one scalar_tensor_tensor
      (ps + (16256-C)) + round(A*bias^T)  -> int16, whose bit pattern IS
      bf16(exp(s+bias)) to ~2%; softmax normalization cancels the uniform
      component, leaving ~1.1e-2 rel error overall (validated in numpy).
  MM2:  po[dv,i] += Vpad_tile^T P^T, ones-column gives denominators free.
PE runs a skew-2 software pipeline (MM1(t+2) issued ahead of MM2(t)) over
3 PSUM score slots so it never waits on ACT/DVE. Warm-up matmuls during
the initial DMA keep the HAM clock gate from throttling the real work.
Host does the final divide/untranspose.
"""

import numpy as np
from contextlib import ExitStack

import concourse.bacc as bacc
import concourse.bass as bass
import concourse.mybir as mybir
import concourse.tile as tile
from concourse.bass_utils import run_bass_kernel_spmd

B = 8          # batch
S = 1024       # sequence length
D = 512        # model dim
H = 8          # heads
HD = 64        # head dim
NT = S // 128  # 128-row j-tiles per sequence
SCALE = HD ** -0.5
A16 = 128.0 / float(np.log(2.0))   # maps scores into bf16 exponent code space
C16 = 7.0                          # Schraudolph correction (tuned numerically)
D_TILES = (1, 3, 5, 7)            # j-tiles exp'd on DVE (Schraudolph)
A_TILES = (0, 2, 4, 6)             # j-tiles exp'd on ACT
N_WARMUP = 8                       # PE warm-up matmuls (HAM clock gate)

_PROGRAM = None


def _emit(ctx, tc, out, qt, kt, vp, eba, abd):
    nc = tc.nc
    f32 = mybir.dt.float32
    bf16 = mybir.dt.bfloat16
    i16 = mybir.dt.int16
    a_slot = {t: i for i, t in enumerate(A_TILES)}
    d_slot = {t: i for i, t in enumerate(D_TILES)}

    singles = ctx.enter_context(tc.tile_pool(name="singles", bufs=1))
    qk_pool = ctx.enter_context(tc.tile_pool(name="qk_pool", bufs=2))
    v_pool = ctx.enter_context(tc.tile_pool(name="v_pool", bufs=2))
    e_pool = ctx.enter_context(tc.tile_pool(name="e_pool", bufs=3))
    # pbf(0) is consumed only at the end of its batch (rotated MM2 order), so
    # keep enough bufs that no same-batch allocation ever WAR-waits on it.
    p_pool = ctx.enter_context(tc.tile_pool(name="p_pool", bufs=10))
    o_pool = ctx.enter_context(tc.tile_pool(name="o_pool", bufs=2))
    ps_s = ctx.enter_context(tc.tile_pool(name="ps_s", bufs=3, space="PSUM"))
    ps_o = ctx.enter_context(tc.tile_pool(name="ps_o", bufs=1, space="PSUM"))

    # First batch's operands on the Sync DMA queue; the resident bias tables
    # on the Activation engine's DMA queue (idle until the first exp) so the
    # ~0.6us/DMA descriptor generation runs in parallel with Sync's.
    qtb = qk_pool.tile([128, S], bf16, tag="qtb")
    ktb = qk_pool.tile([128, S], bf16, tag="ktb")
    vpb = v_pool.tile([128, NT, HD + 1], bf16, tag="vpb")
    nc.sync.dma_start(out=ktb, in_=kt[0])
    nc.sync.dma_start(out=qtb, in_=qt[0])
    nc.sync.dma_start(out=vpb, in_=vp[0])

    # Bias tables: exp(bias^T) bf16 for ACT tiles, round(A*bias^T) int16 for
    # DVE tiles. Partition-major DRAM layout; DMA'd in first-use order
    # (j-tile order 0A 1D 2A 3A 4D 5A 6A 7D) so batch 0 never waits.
    ebt_all = singles.tile([128, len(A_TILES), S], bf16, name="ebt_all")
    abt_all = singles.tile([128, len(D_TILES), S], i16, name="abt_all")
    nc.sync.dma_start(out=ebt_all[:, 0:1, :], in_=eba[:, 0:1, :])
    nc.sync.dma_start(out=abt_all[:, 0:1, :], in_=abd[:, 0:1, :])
    nc.sync.dma_start(out=ebt_all[:, 1:2, :], in_=eba[:, 1:2, :])
    nc.sync.dma_start(out=abt_all[:, 1:2, :], in_=abd[:, 1:2, :])
    nc.sync.dma_start(out=ebt_all[:, 2:, :], in_=eba[:, 2:, :])
    nc.sync.dma_start(out=abt_all[:, 2:, :], in_=abd[:, 2:, :])
    eb_tiles = {t: ebt_all[:, i, :] for i, t in enumerate(A_TILES)}
    ab_tiles = {t: abt_all[:, i, :] for i, t in enumerate(D_TILES)}

    # Warm-up matmuls: keep the PE busy during the DMA head so the HAM
    # clock gate reaches 2.4 GHz before the real matmuls start.
    wrm = singles.tile([128, 512], bf16, name="wrm")
    nc.gpsimd.memset(wrm, 0)
    wps = ps_s.tile([128, S], f32, tag="ps")
    for _ in range(N_WARMUP):
        nc.tensor.matmul(wps[:, 0:512], wrm[:, 0:128], wrm, start=True, stop=True)

    # Per-batch skew-2 software pipeline: MM1(t) | exp(t-1) | MM2(t-2).
    for b in range(B):
        if b > 0:
            qtb = qk_pool.tile([128, S], bf16, tag="qtb", name=f"qtb{b}")
            nc.sync.dma_start(out=qtb, in_=qt[b])
            ktb = qk_pool.tile([128, S], bf16, tag="ktb", name=f"ktb{b}")
            nc.sync.dma_start(out=ktb, in_=kt[b])
            vpb = v_pool.tile([128, NT, HD + 1], bf16, tag="vpb", name=f"vpb{b}")
            nc.sync.dma_start(out=vpb, in_=vp[b])

        po = ps_o.tile([HD + 1, S], f32, tag="po")
        ps_tiles = [None] * NT
        pbf_tiles = [None] * NT
        # Tiles 0 and 2 mul on the slow GpSimd engine, so their MM2s are
        # rotated to the end of the accumulation group (order below); tile 1
        # opens the group (start), tile 2 closes it (stop).
        mm2_order = [1, 3, 4, 5, 6, 7, 0, 2]
        for ph in range(NT + 3):
            # stage 1: scores for tile t
            t = ph
            if t < NT:
                ps = ps_s.tile([128, S], f32, tag="ps")
                ps_tiles[t] = ps
                for c in range(2):
                    cs = slice(c * 512, (c + 1) * 512)
                    nc.tensor.matmul(
                        ps[:, cs],
                        ktb[:, t * 128:(t + 1) * 128],
                        qtb[:, cs],
                        start=True,
                        stop=True,
                    )
            # stage 2: exp for tile u
            u = ph - 1
            if 0 <= u < NT:
                ps = ps_tiles[u]
                pbf = p_pool.tile([128, S], bf16, tag="pbf")
                pbf_tiles[u] = pbf
                if u in d_slot:
                    # pbf bits = int16((ps + 16256-C) + round(A*bias))
                    nc.vector.scalar_tensor_tensor(
                        pbf.bitcast(i16),
                        ps,
                        float(16256.0 - C16),
                        ab_tiles[u],
                        mybir.AluOpType.add,
                        mybir.AluOpType.add,
                    )
                else:
                    ebf = e_pool.tile([128, S], bf16, tag="ebf")
                    nc.scalar.activation(
                        ebf, ps, mybir.ActivationFunctionType.Exp,
                        scale=float(1.0 / A16),
                    )
                    # GpSimd's mul is 2.6x slower than DVE's: it gets tiles
                    # 0 and 2, whose MM2s are deferred to the batch end.
                    eng = nc.gpsimd if u in (0, 2) else nc.vector
                    eng.tensor_mul(pbf, ebf, eb_tiles[u])
            # stage 3: PV accumulation, rotated order
            if ph >= 3:
                w = mm2_order[ph - 3]
                pbf = pbf_tiles[w]
                for c in range(2):
                    cs = slice(c * 512, (c + 1) * 512)
                    nc.tensor.matmul(
                        po[:, cs],
                        vpb[:, w, :],
                        pbf[:, cs],
                        start=(w == 1),
                        stop=(w == 2),
                    )
        osb = o_pool.tile([HD + 1, S], f32, tag="osb")
        nc.scalar.copy(osb, po)
        nc.sync.dma_start(out=out[b], in_=osb)


def _build_program():
    nc = bacc.Bacc("TRN2", target_bir_lowering=False, debug=False)
    qt = nc.dram_tensor("qt", [B, 128, S], mybir.dt.bfloat16, kind="ExternalInput").ap()
    kt = nc.dram_tensor("kt", [B, 128, S], mybir.dt.bfloat16, kind="ExternalInput").ap()
    vp = nc.dram_tensor(
        "vp", [B, 128, NT, HD + 1], mybir.dt.bfloat16, kind="ExternalInput"
    ).ap()
    eba = nc.dram_tensor(
        "eba", [128, len(A_TILES), S], mybir.dt.bfloat16, kind="ExternalInput"
    ).ap()
    abd = nc.dram_tensor(
        "abd", [128, len(D_TILES), S], mybir.dt.int16, kind="ExternalInput"
    ).ap()
    out = nc.dram_tensor("out", [B, HD + 1, S], mybir.dt.float32, kind="ExternalOutput").ap()
    with tile.TileContext(nc) as tc, ExitStack() as ctx:
        _emit(ctx, tc, out, qt, kt, vp, eba, abd)
    nc.compile()
    return nc


def get_program():
    global _PROGRAM
    if _PROGRAM is None:
        _PROGRAM = _build_program()
    return _PROGRAM


def make_in_maps(q, k, v, pos_bias):
    import ml_dtypes

    bf16 = ml_dtypes.bfloat16
    q4 = q.reshape(B, S, H, HD)
    k4 = k.reshape(B, S, H, HD)
    v4 = v.reshape(B, S, H, HD)
    ones = np.ones((B, S, 1), np.float32)
    in_maps = []
    for h in range(H):
        qt = np.zeros((B, 128, S), bf16)
        qt[:, :HD, :] = (
            q4[:, :, h, :].transpose(0, 2, 1) * np.float32(SCALE * A16)
        ).astype(bf16)
        kt = np.zeros((B, 128, S), bf16)
        kt[:, :HD, :] = k4[:, :, h, :].transpose(0, 2, 1).astype(bf16)
        vpf = np.concatenate([v4[:, :, h, :], ones], axis=2)  # [B, S, 65]
        vpf = np.ascontiguousarray(
            vpf.reshape(B, NT, 128, HD + 1).transpose(0, 2, 1, 3)
        ).astype(bf16)  # [B, 128, NT, 65]
        bT = pos_bias[0, h].T  # [j, i]
        # [128(j%128), n_tiles, S(i)] partition-major for single-DMA loads
        eba = np.ascontiguousarray(
            np.stack(
                [np.exp(bT[t * 128:(t + 1) * 128]).astype(bf16) for t in A_TILES]
            ).transpose(1, 0, 2)
        )
        abd = np.ascontiguousarray(
            np.stack(
                [
                    np.rint(bT[t * 128:(t + 1) * 128] * A16).astype(np.int16)
                    for t in D_TILES
                ]
            ).transpose(1, 0, 2)
        )
        in_maps.append({"qt": qt, "kt": kt, "vp": vpf, "eba": eba, "abd": abd})
    return in_maps


def assemble_output(results):
    out = np.empty((B, S, D), np.float32)
    for h in range(H):
        o = results[h]["out"]  # [B, 65, S]
        normed = o[:, :HD, :] / o[:, HD:HD + 1, :]
        out[:, :, h * HD:(h + 1) * HD] = normed.transpose(0, 2, 1)
    return out


def kernel(q, k, v, pos_bias):
    nc = get_program()
    in_maps = make_in_maps(
        np.asarray(q, np.float32),
        np.asarray(k, np.float32),
        np.asarray(v, np.float32),
        np.asarray(pos_bias, np.float32),
    )
    res = run_bass_kernel_spmd(nc, in_maps, list(range(H))).results
    return assemble_output(res)
